# revision 26
# baseline (speedup 1.0000x reference)
"""Trainium2 Bass kernel for per-sample modulated 3x3 conv (StyleGAN2-style).

Math: the reference modulates the shared conv weight per (batch, out-channel)
and demodulates by the weight norm.  Since the modulation is a pure
per-(b,o) scale, the grouped conv factorizes:

    out[b,o] = m[b,o] * conv2d(x[b], W) + conv_bias[o]
    m[b,o]   = eq_c * s[b,o] * rsqrt(s[b,o]^2 * eq_c^2 * q[o] + 1e-8)
    q[o]     = sum_{i,kh,kw} W[o,i,kh,kw]^2
    s[b,o]   = style_w[b] @ (lin_w[o] * eq_l) + lin_b[o] + 1 + mod_bias[o]

so no per-sample weights are ever materialized.  Sharding: data-parallel,
2 samples per core across 8 cores; weights/style params replicated.

The conv itself runs as 1D row-Winograd F(2,3) along the height axis,
cutting the PE matmul count from 576 to 384 per core (the PE stream is
the measured bottleneck at ~222ns per N=512 bf16 matmul):

    rows out (2t, 2t+1) from padded rows d = x[2t..2t+3]:
      V0 = d0-d2, V1 = d1+d2, V2 = d2-d1, V3 = d1-d3      (DVE, bf16)
      U0 = W[kh0], U1 = W[kh0]+W[kh1]+W[kh2],
      U2 = W[kh0]-W[kh1]+W[kh2], U3 = W[kh2]              (DVE, bf16)
      M_xi[o,t,w] = sum_{i,kw} U_xi[i,kw,o] V_xi[i,t,w+kw] (PE, 12 MMs/chain)
      out[2t]   = M0 + M1/2 + M2/2                         (ACT drain + DVE)
      out[2t+1] =      M1/2 - M2/2 - M3                    (ACT drain + DVE)

(The 1/2 of the canonical G matrix is folded into the PSUM drain scale so
U needs only integer adds.)  Per-core schedule notes:
  - sync HWDGE queue: eyeb + x sample-0 + bias columns (8 DMAs); scalar
    queue: eye_f32/style/lin_w constants then x sample 1; gpsimd SWDGE:
    weight fp32->bf16 cast loads chunked per i-block.
  - startup keeps PE/DVE clear for the first chain: style path (own PSUM
    bank) slots in after chain (s0,ot0,xi0); q0 runs as chunked ACT
    Squares after su, q1-3 as 4 small DVE chunks interleaved between
    transpose-copy groups (one big lump on either FIFO stalls the PE via
    PSUM-bank or weight-tile dependencies).
  - weight transposes (identity bf16 matmuls, 3 taps per PSUM bank, DVE
    copy to SBUF) interleave just-in-time inside the chains.
`stage` and `loop_n` are self-test hooks: kernel() builds stage="full",
loop_n=0.
"""

import numpy as np
import ml_dtypes
from math import sqrt

B, C, O, K, LAT, H, W = 16, 512, 512, 3, 256, 32, 32
N_CORES = 8
S = B // N_CORES          # samples per core
P = 128                   # partitions
OT = O // P               # o-tiles
IT = C // P               # i-tiles
KK = K * K                # taps
HP = H + 2                # padded spatial
TY = H // 2               # output row-pairs
NX = 4                    # winograd xi values
LT = LAT // P             # lat-tiles
EQ_CONV = sqrt(2.0 / (C * K * K))
EQ_LIN = sqrt(2.0 / LAT)

_CACHE = {}


def _build(stage="full", loop_n=0):
    import contextlib
    import concourse.bacc as bacc
    import concourse.tile as tile
    from concourse import mybir

    f32 = mybir.dt.float32
    bf16 = mybir.dt.bfloat16
    MUL = mybir.AluOpType.mult
    ADD = mybir.AluOpType.add
    IDENT = mybir.ActivationFunctionType.Identity
    COPY = mybir.ActivationFunctionType.Copy
    SQUARE = mybir.ActivationFunctionType.Square

    nc = bacc.Bacc("TRN2", target_bir_lowering=False, debug=False)

    x_d = nc.dram_tensor("x", [S, C, H, W], f32, kind="ExternalInput").ap()
    sw_d = nc.dram_tensor("style_w", [S, LAT], f32, kind="ExternalInput").ap()
    cw_d = nc.dram_tensor("conv_weight", [O, C, K, K], f32, kind="ExternalInput").ap()
    lw_d = nc.dram_tensor("lin_w", [O, LAT], f32, kind="ExternalInput").ap()
    lb_d = nc.dram_tensor("lin_b", [O], f32, kind="ExternalInput").ap()
    mb_d = nc.dram_tensor("mod_bias", [O], f32, kind="ExternalInput").ap()
    cb_d = nc.dram_tensor("conv_bias", [O], f32, kind="ExternalInput").ap()
    eyeb_d = nc.dram_tensor("eye_bf", [P, P], bf16, kind="ExternalInput").ap()
    eyef_d = nc.dram_tensor("eye_f32", [P, P], f32, kind="ExternalInput").ap()
    out_d = nc.dram_tensor("out", [S, O, H, W], f32, kind="ExternalOutput").ap()

    cw2 = cw_d.rearrange("o i kh kw -> o (i kh kw)")          # [512, 4608]
    lb2 = lb_d.rearrange("(t p) -> p t", p=P)                 # [128, 4] strided
    mb2 = mb_d.rearrange("(t p) -> p t", p=P)
    cb2 = cb_d.rearrange("(t p) -> p t", p=P)

    lvl = {"dma": 1, "style": 2, "qm": 3, "transpose": 4, "full": 5}[stage]

    with tile.TileContext(nc) as tc:
        loop = tc.For_i(0, loop_n, 1) if loop_n else contextlib.nullcontext()
        with (
            loop,
            tc.tile_pool(name="const", bufs=1) as cp,
            tc.tile_pool(name="xf32", bufs=4) as xp,
            tc.tile_pool(name="xpadp", bufs=2) as xp2,
            tc.tile_pool(name="work", bufs=2) as wp,
            tc.tile_pool(name="outp", bufs=2) as op_,
            tc.tile_pool(name="wnp", bufs=2) as wnp,
            tc.tile_pool(name="utp", bufs=2) as utp,
            tc.tile_pool(name="cpsum", bufs=4, space="PSUM") as pconv,
            tc.tile_pool(name="tpsum", bufs=3, space="PSUM") as ptp,
            tc.tile_pool(name="spsum", bufs=1, space="PSUM") as psty,
        ):
            # ---------- sync queue: EXACTLY 8 DMAs (eyeb, x-s0 x4,
            # biases x3) so x lands as early as possible; the style
            # constants ride the otherwise-idle scalar queue ----------
            eyeb = cp.tile([P, P], bf16, tag="eyeb")
            nc.sync.dma_start(eyeb[:], eyeb_d[:])
            eyef = cp.tile([P, P], f32, tag="eyef")
            nc.scalar.dma_start(eyef[:], eyef_d[:])
            style_sb = cp.tile([S, LAT], f32, tag="style")
            nc.scalar.dma_start(style_sb[:], sw_d[:])
            linwt = cp.tile([P, OT * LAT], f32, tag="linwt")
            nc.scalar.dma_start(linwt[:].rearrange("p (t l) -> p t l", t=OT),
                                lw_d.rearrange("(t p) l -> p t l", p=P))
            linw = [linwt[:, ot * LAT:(ot + 1) * LAT] for ot in range(OT)]

            xpad = {}
            # winograd row-transformed input: [128, (xi, ty, 34)] bf16
            vt = [[cp.tile([P, NX * TY * HP], bf16, tag=f"v{s}_{it}",
                           name=f"v{s}_{it}") for it in range(IT)]
                  for s in range(S)]
            xf32 = [[None] * IT for _ in range(S)]

            def emit_x_dma(s, eng):
                for it in range(IT):
                    t = xp.tile([P, H * W], f32, tag="xf32", name="xf32")
                    eng.dma_start(
                        t[:].rearrange("p (h w) -> p h w", w=W),
                        x_d[s, it * P:(it + 1) * P, :, :])
                    xf32[s][it] = t

            def emit_x_cast(s, it):
                t = xp2.tile([P, HP, HP], bf16, tag=f"xpad{it}", name="xpad")
                xpad[s, it] = t
                nc.vector.memset(t[:, 0:1, :], 0.0)
                nc.vector.memset(t[:, HP - 1:HP, :], 0.0)
                nc.vector.memset(t[:, 1:H + 1, 0:HP:HP - 1], 0.0)
                nc.scalar.activation(
                    t[:, 1:H + 1, 1:W + 1],
                    xf32[s][it][:].rearrange("p (h w) -> p h w", w=W),
                    COPY)

            def emit_v(s, it):
                """row-Winograd input transform on DVE (4 tensor ops)."""
                xp3 = xpad[s, it]
                r0 = xp3[:, 0:32:2, :]
                r1 = xp3[:, 1:33:2, :]
                r2 = xp3[:, 2:34:2, :]
                r3 = xp3[:, 3:34:2, :]
                v4 = vt[s][it][:].rearrange("p (f t w) -> p f t w", f=NX, w=HP)
                nc.vector.tensor_sub(v4[:, 0], r0, r2)
                nc.vector.tensor_add(v4[:, 1], r1, r2)
                nc.vector.tensor_sub(v4[:, 2], r2, r1)
                nc.vector.tensor_sub(v4[:, 3], r1, r3)

            emit_x_dma(0, nc.sync)
            # per-o-tile bias columns: 4B-strided descriptor-sludge DMAs,
            # parked at the queue tail where nothing waits behind them
            lbt = cp.tile([P, OT], f32, tag="lbt")
            nc.sync.dma_start(lbt[:], lb2)
            mbt = cp.tile([P, OT], f32, tag="mbt")
            nc.sync.dma_start(mbt[:], mb2)
            cbt = cp.tile([P, OT], f32, tag="cbt")
            nc.sync.dma_start(cbt[:], cb2)

            # ---------- gpsimd (SWDGE): only the weight cast loads, in
            # contiguous per-i-block chunks ----------
            wnat = []
            for ot in range(OT):
                t = wnp.tile([P, C * KK], bf16, tag="wnat", name="wnat")
                wnat.append(t)
                for it in range(IT):
                    lo = it * P * KK
                    hi = (it + 1) * P * KK
                    nc.gpsimd.dma_start(t[:, lo:hi],
                                        cw2[ot * P:(ot + 1) * P, lo:hi])

            if lvl < 5:
                for s in range(S):
                    for ot in range(OT):
                        ob = op_.tile([P, H * W], f32, tag="ob", name="ob")
                        nc.vector.memset(ob[:], 0.0)
                        nc.sync.dma_start(
                            out_d[s, ot * P:(ot + 1) * P, :, :],
                            ob[:].rearrange("p (h w) -> p h w", w=W),
                        )

            # ---------- style path (fp32 on PE; sign of s matters) ----
            su = []

            def emit_style():
                if lvl < 2:
                    return
                styleT = []
                for lt in range(LT):
                    ps = psty.tile([P, S], f32, tag="sp", name="sp")
                    nc.tensor.transpose(ps[:], style_sb[:, lt * P:(lt + 1) * P],
                                        eyef[0:S, 0:S])
                    t = cp.tile([P, S], f32, tag=f"styleT{lt}")
                    nc.vector.tensor_copy(t[:], ps[:])
                    styleT.append(t)
                linwT = [[None] * OT for _ in range(LT)]
                for ot in range(OT):
                    for lt in range(LT):
                        ps = psty.tile([P, P], f32, tag="sp", name="sp")
                        nc.tensor.transpose(ps[:],
                                            linw[ot][:, lt * P:(lt + 1) * P],
                                            eyef[:])
                        t = cp.tile([P, P], f32, tag=f"linwT{lt}_{ot}")
                        nc.vector.tensor_copy(t[:], ps[:])
                        linwT[lt][ot] = t
                bv2 = []
                for ot in range(OT):
                    bv = wp.tile([P, 1], f32, tag="bv")
                    nc.vector.tensor_add(bv[:], lbt[:, ot:ot + 1],
                                         mbt[:, ot:ot + 1])
                    t = cp.tile([P, 1], f32, tag=f"bv2{ot}")
                    nc.vector.tensor_scalar_add(t[:], bv[:], 1.0)
                    bv2.append(t)
                # su[ot] = (sum_lt linwT.T @ styleT) * eq_lin + bvec
                for ot in range(OT):
                    ps = psty.tile([P, S], f32, tag="sp", name="sp")
                    for lt in range(LT):
                        nc.tensor.matmul(ps[:], linwT[lt][ot][:], styleT[lt][:],
                                         start=(lt == 0), stop=(lt == LT - 1))
                    t = cp.tile([P, S], f32, tag=f"su{ot}")
                    nc.scalar.activation(t[:], ps[:], IDENT, bias=bv2[ot][:],
                                         scale=EQ_LIN)
                    su.append(t)

            # ---------- winograd weight planes + transposed tiles ------
            # u1 = W[kh0]+W[kh1]+W[kh2], u2 = W[kh0]-W[kh1]+W[kh2]
            # (integer G; the 1/2 is folded into the M1/M2 PSUM drains)
            u1, u2, ut = {}, {}, {}

            def get_ut(ot):
                if ot not in ut:
                    ut[ot] = utp.tile([P, NX * K * IT * P], bf16, tag="ut",
                                      name="ut")
                return ut[ot]

            mcol = [None] * OT

            def emit_umix(ot):
                """u1/u2 planes on DVE (3 tensor ops over [128,(i,kw)])."""
                if lvl < 4:
                    return
                wn4 = wnat[ot][:].rearrange("p (i kh kw) -> p i kh kw", kh=K,
                                            kw=K)
                ts = wp.tile([P, C * K], bf16, tag="umixt", name="umixt")
                t3 = ts[:].rearrange("p (i kw) -> p i kw", kw=K)
                nc.vector.tensor_add(t3[:], wn4[:, :, 0, :], wn4[:, :, 2, :])
                u1[ot] = wp.tile([P, C * K], bf16, tag="u1", name="u1")
                u13 = u1[ot][:].rearrange("p (i kw) -> p i kw", kw=K)
                nc.vector.tensor_add(u13[:], t3[:], wn4[:, :, 1, :])
                u2[ot] = wp.tile([P, C * K], bf16, tag="u2", name="u2")
                u23 = u2[ot][:].rearrange("p (i kw) -> p i kw", kw=K)
                nc.vector.tensor_sub(u23[:], t3[:], wn4[:, :, 1, :])

            def emit_q(ot, act_chunked=False):
                """q[ot] = sum W^2 without poisoning any FIFO."""
                if lvl < 3:
                    return None
                if act_chunked:
                    qp = []
                    for j in range(IT):
                        qscr = wp.tile([P, P * KK], bf16, tag="qscr", name="qscr")
                        t = wp.tile([P, 1], f32, tag=f"qp{j}", name="qp")
                        nc.scalar.activation(
                            qscr[:], wnat[ot][:, j * P * KK:(j + 1) * P * KK],
                            SQUARE, accum_out=t[:])
                        qp.append(t)
                    return emit_q_fini(ot, qp)
                qp = []
                for j in range(IT):
                    emit_q_dve_chunk(ot, j, qp)
                return emit_q_fini(ot, qp)

            def emit_q_dve_chunk(ot, j, qparts):
                """One quarter of q[ot] on DVE: keeps each DVE lump under
                ~2us so interleaved transpose copies are never starved."""
                if lvl < 3:
                    return
                lo, hi = j * P * KK, (j + 1) * P * KK
                qscr = wp.tile([P, P * KK], bf16, tag="qscr", name="qscr")
                nc.vector.tensor_mul(qscr[:], wnat[ot][:, lo:hi],
                                     wnat[ot][:, lo:hi])
                t = wp.tile([P, 1], f32, tag=f"qp{j}", name="qp")
                nc.vector.tensor_reduce(t[:], qscr[:], mybir.AxisListType.X, ADD)
                qparts.append(t)

            def emit_q_fini(ot, qparts):
                if lvl < 3:
                    return None
                a = wp.tile([P, 1], f32, tag="qa", name="qa")
                nc.vector.tensor_add(a[:], qparts[0][:], qparts[1][:])
                b = wp.tile([P, 1], f32, tag="qb", name="qb")
                nc.vector.tensor_add(b[:], qparts[2][:], qparts[3][:])
                qv = cp.tile([P, 1], f32, tag=f"q{ot}")
                nc.vector.tensor_add(qv[:], a[:], b[:])
                return qv

            def emit_m(ot, qv):
                """m[ot] = u*rsqrt(u^2*q+1e-8): DVE ops + one ACT sqrt."""
                if lvl < 3:
                    return
                uv = cp.tile([P, S], f32, tag=f"uv{ot}")
                nc.vector.tensor_scalar_mul(uv[:], su[ot][:], EQ_CONV)
                u2_ = wp.tile([P, S], f32, tag="usq")
                nc.vector.tensor_mul(u2_[:], uv[:], uv[:])
                vv = wp.tile([P, S], f32, tag="vv", name="vv")
                nc.vector.tensor_scalar(vv[:], u2_[:], qv[:], 1e-8, MUL, ADD)
                sq = wp.tile([P, S], f32, tag="sq", name="sq")
                nc.scalar.sqrt(sq[:], vv[:])
                rv = wp.tile([P, S], f32, tag="rv")
                nc.vector.reciprocal(rv[:], sq[:])
                mv = cp.tile([P, S], f32, tag=f"m{ot}")
                nc.vector.tensor_mul(mv[:], uv[:], rv[:])
                mcol[ot] = mv

            def emit_transpose(ot, groups):
                """Identity-matmul transposes; one group = 3 kw taps of one
                (it, xi) into one PSUM bank + one batched DVE copy."""
                if lvl < 4:
                    return
                wn4 = wnat[ot][:].rearrange("p (i kh kw) -> p i kh kw", kh=K,
                                            kw=K)
                u13 = u1[ot][:].rearrange("p (i kw) -> p i kw", kw=K)
                u23 = u2[ot][:].rearrange("p (i kw) -> p i kw", kw=K)
                ut4 = get_ut(ot)[:].rearrange("p (t x o) -> p t x o",
                                              t=NX * K, x=IT)
                for gi in groups:
                    it, xi = divmod(gi, NX)
                    ib = slice(it * P, (it + 1) * P)
                    ps = ptp.tile([P, K * P], f32, tag="tp", name="tp")
                    for kw in range(K):
                        if xi == 0:
                            src = wn4[:, ib, 0, kw]
                        elif xi == 1:
                            src = u13[:, ib, kw]
                        elif xi == 2:
                            src = u23[:, ib, kw]
                        else:
                            src = wn4[:, ib, 2, kw]
                        nc.tensor.matmul(ps[:, kw * P:(kw + 1) * P], src,
                                         eyeb[:], start=True, stop=True)
                    nc.vector.tensor_copy(
                        ut4[:, xi * K:(xi + 1) * K, it, :],
                        ps[:].rearrange("p (k o) -> p k o", k=K))

            def emit_chain_mms(s, ot, xi, pre=None):
                """12 matmuls (it x kw) accumulating M_xi into one PSUM bank.
                `pre` maps i-block -> emit-thunks run before that block."""
                ut4 = get_ut(ot)[:].rearrange("p (t x o) -> p t x o",
                                              t=NX * K, x=IT)
                cps = pconv.tile([P, TY * W], f32, tag="cps", name="cps")
                for it in range(IT):
                    if pre is not None:
                        for fn in pre.get(it, []):
                            fn()
                    v4 = vt[s][it][:].rearrange("p (f t w) -> p f t w", f=NX,
                                                w=HP)
                    for kw in range(K):
                        rhs = v4[:, xi, :, kw:kw + W]
                        nc.tensor.matmul(
                            cps[:], ut4[:, xi * K + kw, it, :], rhs,
                            start=(it == 0 and kw == 0),
                            stop=(it == IT - 1 and kw == K - 1))
                return cps

            def emit_out(s, ot, cps_list):
                """ACT drains (x0.5 on M1/M2) -> DVE row combos -> ACT
                epilogue (scale m, bias) into interleaved rows -> store."""
                md = []
                for xi in range(NX):
                    t = op_.tile([P, TY * W], bf16, tag=f"md{xi}",
                                 name=f"md{xi}")
                    nc.scalar.activation(t[:], cps_list[xi][:], COPY,
                                         scale=(0.5 if xi in (1, 2) else 1.0))
                    md.append(t)
                t0 = wp.tile([P, TY * W], bf16, tag="y0a", name="y0a")
                nc.vector.tensor_add(t0[:], md[0][:], md[1][:])
                y0 = wp.tile([P, TY * W], bf16, tag="y0", name="y0")
                nc.vector.tensor_add(y0[:], t0[:], md[2][:])
                t1 = wp.tile([P, TY * W], bf16, tag="y1a", name="y1a")
                nc.vector.tensor_sub(t1[:], md[1][:], md[2][:])
                y1 = wp.tile([P, TY * W], bf16, tag="y1", name="y1")
                nc.vector.tensor_sub(y1[:], t1[:], md[3][:])
                ob = op_.tile([P, H * W], f32, tag="ob", name="ob")
                ob3 = ob[:].rearrange("p (h w) -> p h w", w=W)
                for d, y in ((0, y0), (1, y1)):
                    nc.scalar.activation(
                        ob3[:, d:H:2, :],
                        y[:].rearrange("p (t w) -> p t w", w=W),
                        IDENT, bias=cbt[:, ot:ot + 1],
                        scale=mcol[ot][:, s:s + 1])
                nc.scalar.dma_start(out_d[s, ot * P:(ot + 1) * P, :, :], ob3)

            def emit_block(s, ot, pre=None):
                """The 4 xi-chains of one (s, ot) + output transform."""
                cps_list = []
                for xi in range(NX):
                    p = pre.get(xi, None) if pre else None
                    cps_list.append(emit_chain_mms(s, ot, xi, pre=p))
                emit_out(s, ot, cps_list)

            def tr(ot, gis):
                return lambda: emit_transpose(ot, gis)

            # ---------- orchestration: blocks run (s0,ot),(s1,ot) pairs
            # so each o-tile's DVE prep (umix + 16 transpose copies + q
            # chunks ~21us) is covered by ~21us of PE work; transposes
            # and q chunks drop just-in-time into the chain pre-slots ---
            def prep_pre(ot, qp):
                return {
                    0: {it: [tr(ot, [it * NX])] for it in range(IT)},
                    1: {it: [tr(ot, [it * NX + 1]),
                             (lambda o=ot, j=it, q=qp:
                              emit_q_dve_chunk(o, j, q))]
                        for it in range(IT)},
                    2: {it: [tr(ot, [it * NX + 2] +
                                ([it * NX + 3] if it % 2 == 0 else []))]
                        for it in range(IT)},
                    3: {it: [tr(ot, [it * NX + 3])]
                        for it in range(1, IT, 2)},
                }

            if lvl >= 5:
                # casts + V build, interleaved per i-block so the first
                # chain's it-block inputs appear in stream order
                for it in range(IT):
                    emit_x_cast(0, it)
                    emit_v(0, it)
                emit_x_dma(1, nc.scalar)
                emit_umix(0)
                pre0 = prep_pre(0, [])
                pre0[1] = {it: [tr(0, [it * NX + 1])] for it in range(IT)}
                cps_list0 = [emit_chain_mms(0, 0, 0, pre=pre0[0])]
                emit_style()
                qv0 = emit_q(0, act_chunked=True)
                emit_m(0, qv0)
                for xi in range(1, NX):
                    cps_list0.append(emit_chain_mms(0, 0, xi, pre=pre0[xi]))
                emit_out(0, 0, cps_list0)
                for it in range(IT):
                    emit_x_cast(1, it)
                vpre = {0: {it: [(lambda s_=1, j=it: emit_v(s_, j))]
                            for it in range(IT)}}
                emit_block(1, 0, pre=vpre)
                for ot in range(1, OT):
                    qp = []
                    pre = prep_pre(ot, qp)
                    emit_umix(ot)
                    cps_list = [emit_chain_mms(0, ot, 0, pre=pre[0])]
                    cps_list.append(emit_chain_mms(0, ot, 1, pre=pre[1]))
                    emit_m(ot, emit_q_fini(ot, qp))
                    for xi in range(2, NX):
                        cps_list.append(emit_chain_mms(0, ot, xi, pre=pre[xi]))
                    emit_out(0, ot, cps_list)
                    emit_block(1, ot)
            else:
                for it in range(IT):
                    emit_x_cast(0, it)
                    emit_v(0, it)
                emit_x_dma(1, nc.scalar)
                for it in range(IT):
                    emit_x_cast(1, it)
                    emit_v(1, it)
                emit_style()
                for ot in range(OT):
                    emit_umix(ot)
                    emit_transpose(ot, range(NX * IT))
                    emit_m(ot, emit_q(ot))

    nc.compile()
    return nc


def kernel(x, style_w, conv_weight, lin_w, lin_b, mod_bias, conv_bias):
    from concourse.bass_utils import run_bass_kernel_spmd

    if "nc" not in _CACHE:
        _CACHE["nc"] = _build()
    nc = _CACHE["nc"]

    eye_bf = np.eye(P, dtype=ml_dtypes.bfloat16)
    eye_f32 = np.eye(P, dtype=np.float32)
    x = np.ascontiguousarray(np.asarray(x, dtype=np.float32))
    style_w = np.ascontiguousarray(np.asarray(style_w, dtype=np.float32))
    conv_weight = np.ascontiguousarray(np.asarray(conv_weight, dtype=np.float32))
    lin_w = np.ascontiguousarray(np.asarray(lin_w, dtype=np.float32))
    lin_b = np.ascontiguousarray(np.asarray(lin_b, dtype=np.float32))
    mod_bias = np.ascontiguousarray(np.asarray(mod_bias, dtype=np.float32))
    conv_bias = np.ascontiguousarray(np.asarray(conv_bias, dtype=np.float32))

    in_maps = []
    for c in range(N_CORES):
        in_maps.append({
            "x": x[c * S:(c + 1) * S],
            "style_w": style_w[c * S:(c + 1) * S],
            "conv_weight": conv_weight,
            "lin_w": lin_w,
            "lin_b": lin_b,
            "mod_bias": mod_bias,
            "conv_bias": conv_bias,
            "eye_bf": eye_bf,
            "eye_f32": eye_f32,
        })
    res = run_bass_kernel_spmd(nc, in_maps, list(range(N_CORES)))
    return np.concatenate([res.results[c]["out"] for c in range(N_CORES)], axis=0)
